# revision 14
# baseline (speedup 1.0000x reference)
"""Trainium2 Bass kernel for nn_DenseGNOBlock (B=4, N=8192, C=64).

Reference computes, per batch b:
    q = x Wq^T + bq ; k = x Wk^T + bk ; v = x Wv^T + bv
    kernel = q k^T / sqrt(C) ; integral = kernel v / N
    out = gelu(x Ww^T + bw + integral)

No softmax, so the N x N kernel reassociates away completely. With the
ones-FIRST augmentation Xa = [1|x] (N x 65) and Wt* = [b*; W*^T] (65 x 64):
    Gt  = Xa^T Xa                         (65 x 65, symmetric)
    Mt  = Wtw + a Wtq Wtk^T Gt Wtv        (a = 1/(sqrt(C) N))
    out = gelu(Xa @ Mt)
The rest is precision + layout engineering against the cost model:

- Gt's body accumulates from an fp8(e4m3) copy of x with DoubleRow
  matmuls: each instruction contracts TWO 128-row groups at 0.5 PE
  cycles/row, so the 8192-row Gram fits in 32 matmuls. The dual-fp8
  ldweights ISA check demands <=128 weight columns at an even,
  16B-aligned pair stride, so x ships in 80-col padded groups
  [1|x(64)|zeros(15)]; the ones column rides only the rhs (moving
  side), making each matmul yield [m | G] = Gt rows 1:65 into one
  PSUM tile. Gt's quantization noise averages out over N=8192: end-to-
  end rel err ~3e-3 (tolerance 2e-2).
- Gt's missing top row never gets materialized: the chain computes
  T1 = Gt Wtv in permuted row order (body first) from [m | G] alone --
  G Wv^T via symmetry, the m bv^T term through diag(m) (a per-partition
  tensor_scalar of the identity; no transposes anywhere), and T1's
  own-row via two 1-row matmuls; U's columns are host-permuted to
  match.
- Everything else is bf16 (PSUM accumulation stays fp32): 1 PE
  cycle/row instead of fp32's 4.
- x also ships pre-TRANSPOSED in bf16 (xt, own half only) for the
  output matmuls. Its column order bakes in the quad-interleaved row
  permutation that makes the bf16 output DMA 512B-contiguous.
- DMAs spread over the three DMA-capable rings (SP, gpsimd/SWDGE, and
  ACT for the weight pack behind its activation-table load). gelu
  reads PSUM in 3 ops sized [8,16,8] chunks: big enough that the fixed
  access latency doesn't dominate, staged so ACT never idles between
  the first matmul group and the last out-DMA (which ACT itself
  issues, keeping the slow SWDGE ring out of the drain tail).

Sharding: 8 cores, core c -> batch b = c//2, half h = c%2. Each core
receives the full x_b (for Gt) + its own transposed half, writes its half.
"""

import sys

for _p in ("/opt/trn_rl_repo", "/root/.axon_site/_ro/trn_rl_repo"):
    if _p not in sys.path:
        sys.path.append(_p)

import numpy as np
import ml_dtypes
from contextlib import ExitStack

import concourse.bass as bass
import concourse.bacc as bacc
import concourse.mybir as mybir
import concourse.tile as tile
from concourse.bass_utils import run_bass_kernel_spmd

FP = mybir.dt.float32
BF = mybir.dt.bfloat16
F8 = mybir.dt.float8e4
AF = mybir.ActivationFunctionType
DR = mybir.MatmulPerfMode.DoubleRow
BF_NP = ml_dtypes.bfloat16
F8_NP = ml_dtypes.float8_e4m3

B, N, C = 4, 8192, 64
P = 128               # partitions
W = C + 1             # augmented width (ones-first)
GW = 80               # padded group width: [1|x(64)|0(15)], 16B-aligned
NBLK = 8              # oct row-blocks per batch (1024 rows each)
BCOL = 8 * GW         # 640 cols per block
NH = N // 2           # own half rows
NCH = NH // P         # 32 final-matmul chunks
NCORES = 8
ALPHA = 1.0 / (np.sqrt(np.float32(C)) * np.float32(N))
# packed weight layout (free offsets in wpk [65, WPK_F])
WPK_UT = 0            # [0:65, 0:65]     (U~)^T, U~ = U cols rotated [1..64,0]
WPK_WB = W            # [0:65, 65:129]   Wtw = [bw; Ww^T]
WPK_WV = W + C        # [0:64, 129:193]  Wv^T
WPK_BV = W + 2 * C    # [0:64, 193:257]  ones x bv^T
WPK_ID = W + 3 * C    # [0:64, 257:321]  I64
WPK_NB = W + 4 * C    # [0:1, 321:385]   N * bv^T ; [0:1, 385] = 1.0
WPK_F = W + 5 * C + 1  # 386
G_ORDER = [0, 4, 5, 6, 7, 1, 2, 3]   # follows DMA arrival order
OGRP = [(0, 8), (8, 24), (24, 32)]   # gelu groups (chunk ranges, 4-aligned)


def build_nc(act: str = "gelu") -> bass.Bass:
    act_fn = {"gelu": AF.Gelu, "identity": AF.Identity}[act]
    nc = bacc.Bacc("TRN2", target_bir_lowering=False, debug=False)

    xq_d = nc.declare_dram_parameter("xq", [P, NBLK * BCOL], F8, isOutput=False)
    xt_d = nc.declare_dram_parameter("xt", [W, NH], BF, isOutput=False)
    wpk_d = nc.declare_dram_parameter("wpk", [W, WPK_F], BF, isOutput=False)
    out_d = nc.declare_dram_parameter("out", [NH, C], BF, isOutput=True)

    with ExitStack() as ctx:
        tc = ctx.enter_context(tile.TileContext(nc))
        const = ctx.enter_context(tc.tile_pool(name="const", bufs=1))
        ps_g = ctx.enter_context(tc.tile_pool(name="ps_g", bufs=2, space="PSUM"))
        ps_o = ctx.enter_context(tc.tile_pool(name="ps_o", bufs=1, space="PSUM"))

        wpk = const.tile([W, WPK_F], BF)
        ut = wpk[:, WPK_UT : WPK_UT + W]
        wwb = wpk[:, WPK_WB : WPK_WB + C]
        wvs = wpk[0:C, WPK_WV : WPK_WV + C]
        bvb = wpk[0:C, WPK_BV : WPK_BV + C]
        id64 = wpk[0:C, WPK_ID : WPK_ID + C]
        nbv = wpk[0:1, WPK_NB : WPK_NB + C]
        one1 = wpk[0:1, WPK_NB + C : WPK_NB + C + 1]
        xqs = const.tile([P, NBLK, 8, GW], F8)
        xts = const.tile([W, NH], BF)

        # --- input DMAs: three rings issue in parallel ------------------
        # first block ships alone so the Gram matmuls start at the DMA
        # round-trip floor; xt trails x on both rings (needed ~1.5us later)
        xqr = xq_d[:].rearrange("p (b k w) -> p b k w", k=8, w=GW)
        nc.sync.dma_start(out=xqs[:, 0:1], in_=xqr[:, 0:1])
        nc.sync.dma_start(out=xqs[:, 1:4], in_=xqr[:, 1:4])
        nc.gpsimd.dma_start(out=xqs[:, 4:6], in_=xqr[:, 4:6])
        nc.gpsimd.dma_start(out=xqs[:, 6:8], in_=xqr[:, 6:8])
        nc.scalar.dma_start(out=wpk[:], in_=wpk_d[:])
        nc.sync.dma_start(out=xts[:, 0 : NH // 2], in_=xt_d[:, 0 : NH // 2])
        nc.gpsimd.dma_start(out=xts[:, NH // 2 :], in_=xt_d[:, NH // 2 :])

        # --- Gt body accumulation: one PSUM tile, 32 DoubleRow matmuls --
        # lhsT = two x groups (128 weight cols, stride 80 = 16B-aligned);
        # rhs keeps the ones cols -> out accumulates [m | G] (Gt rows 1:65)
        gt_ps = ps_g.tile([C, W], FP, tag="chain")
        nmm = NBLK * 4
        i = 0
        for blk in G_ORDER:
            for j in range(4):
                pair = xqs[:, blk, 2 * j : 2 * j + 2]
                nc.tensor.matmul(
                    gt_ps[:], pair[:, :, 1 : 1 + C], pair[:, :, 0:W],
                    perf_mode=DR, start=(i == 0), stop=(i == nmm - 1),
                )
                i += 1

        # --- chain: T1 = Gt Wtv (row-permuted, body first) ; Mt --------
        gts = const.tile([C, W], BF)
        # ACT is idle here and Identity shares Gelu's table (gelu_and_others):
        # both chain copies ride ACT in parallel with the DVE diag(m) path
        nc.scalar.activation(gts[:], gt_ps[:], AF.Identity)
        msb = gts[:, 0:1]
        m32 = const.tile([C, 1], FP)      # fp32 twin: tensor_scalar wants it
        nc.vector.tensor_copy(m32[:], gt_ps[:, 0:1])
        diagm = const.tile([C, C], BF)
        nc.vector.tensor_scalar_mul(diagm[:], id64, m32[:])
        # T1 body = G Wv^T + m bv^T (G via symmetry, m bv^T via diag(m))
        t1b_ps = ps_g.tile([C, C], FP, tag="chain")
        nc.tensor.matmul(t1b_ps[:], gts[:, 1:W], wvs, start=True, stop=False)
        nc.tensor.matmul(t1b_ps[:], diagm[:], bvb, start=False, stop=True)
        # T1 top row = m^T Wv^T + N bv^T  (lands at permuted position 64)
        t1r_ps = ps_g.tile([1, C], FP, tag="r0")
        nc.tensor.matmul(t1r_ps[:], msb, wvs, start=True, stop=False)
        nc.tensor.matmul(t1r_ps[:], one1, nbv, start=False, stop=True)
        t1s = const.tile([W, C], BF)
        nc.vector.tensor_copy(t1s[0:C, :], t1b_ps[:])
        nc.scalar.activation(t1s[C : C + 1, :], t1r_ps[:], AF.Identity)
        acr_ps = ps_g.tile([W, C], FP, tag="chain")
        nc.tensor.matmul(acr_ps[:], ut, t1s[:])
        mts = const.tile([W, C], BF)
        nc.vector.tensor_add(mts[:], acr_ps[:], wwb)

        # --- own half: out = gelu(Xa Mt), gelu straight from PSUM -------
        # chunk c covers own rows 512*(c//4) + 4p + (c%4) (baked into xt's
        # column order), so a 4-aligned chunk group [c0,c1) maps to the
        # contiguous HBM row range [128*c0, 128*c1) with 512B runs
        osb = const.tile([P, NCH // 4, 4, C], BF)
        for gi, (c0, c1) in enumerate(OGRP):
            ng = c1 - c0
            po = ps_o.tile([P, ng, C], FP, tag=f"po{gi}")
            for k in range(ng):
                ch = c0 + k
                nc.tensor.matmul(
                    po[:, k, :], xts[:, ch * P : (ch + 1) * P], mts[:]
                )
            og = osb[:, c0 // 4 : c1 // 4]
            nc.scalar.activation(
                og.rearrange("p g j c -> p (g j c)"),
                po[:].rearrange("p a c -> p (a c)"),
                act_fn,
            )
            orr = out_d[128 * c0 : 128 * c1].rearrange(
                "(g p j) c -> p g j c", p=P, j=4
            )
            # last group's DMA issues from ACT right behind its own gelu;
            # earlier groups ride SP (the slower SWDGE ring would add its
            # bigger completion latency to the drain tail)
            eng = (nc.sync, nc.sync, nc.scalar)[gi]
            eng.dma_start(out=orr, in_=og)

    nc.compile()
    return nc


_NC_CACHE = None


def _get_nc() -> bass.Bass:
    global _NC_CACHE
    if _NC_CACHE is None:
        _NC_CACHE = build_nc()
    return _NC_CACHE


def make_wpk(inputs: dict) -> np.ndarray:
    Wq, Wk, Wv, Ww = (np.asarray(inputs[k], np.float32) for k in ("Wq", "Wk", "Wv", "Ww"))
    bq, bk, bv, bw = (np.asarray(inputs[k], np.float32) for k in ("bq", "bk", "bv", "bw"))
    wtq = np.concatenate([bq[None, :], Wq.T], axis=0)   # [65, 64]
    wtk = np.concatenate([bk[None, :], Wk.T], axis=0)
    wwb = np.concatenate([bw[None, :], Ww.T], axis=0)
    u = (ALPHA * (wtq @ wtk.T)).astype(np.float32)      # [65, 65]
    uperm = u[:, list(range(1, W)) + [0]]               # cols body-first
    wpk = np.zeros((W, WPK_F), np.float32)
    wpk[:, WPK_UT : WPK_UT + W] = uperm.T
    wpk[:, WPK_WB : WPK_WB + C] = wwb
    wpk[0:C, WPK_WV : WPK_WV + C] = Wv.T
    wpk[0:C, WPK_BV : WPK_BV + C] = bv[None, :]
    wpk[0:C, WPK_ID : WPK_ID + C] = np.eye(C, dtype=np.float32)
    wpk[0, WPK_NB : WPK_NB + C] = np.float32(N) * bv
    wpk[0, WPK_NB + C] = 1.0
    return wpk.astype(BF_NP)


def make_in_maps(inputs: dict) -> list[dict]:
    x = np.asarray(inputs["x"], dtype=np.float32)        # [B, N, C]
    x16 = x.astype(BF_NP)
    x8 = x.astype(F8_NP)
    wpk = np.ascontiguousarray(make_wpk(inputs))
    in_maps = []
    for core in range(NCORES):
        b, h = core // 2, core % 2
        # xq oct-blocks (fp8): row(blk, p, k) = 1024 blk + 8 p + k;
        # group layout [1 | x | 0*15] keeps the DoubleRow pair stride
        # 16B-aligned and the DMA runs 640B-contiguous
        xq = np.zeros((P, NBLK, 8, GW), F8_NP)
        xq[:, :, :, 0] = 1.0
        xq[:, :, :, 1 : 1 + C] = x8[b].reshape(NBLK, P, 8, C).transpose(1, 0, 2, 3)
        # xt own half (bf16), transposed, quad-interleaved column order:
        # column 128 c + p  <->  own row 512 (c//4) + 4 p + (c%4)
        xo = x16[b, h * NH : (h + 1) * NH]               # [4096, 64]
        xt = np.ones((W, NH), BF_NP)
        xt[1:] = (
            xo.reshape(NH // 512, P, 4, C)               # [g, p, j, c]
            .transpose(3, 0, 2, 1)                       # [c, g, j, p]
            .reshape(C, NH)
        )
        in_maps.append(
            dict(
                xq=np.ascontiguousarray(xq.reshape(P, NBLK * BCOL)),
                xt=np.ascontiguousarray(xt),
                wpk=wpk,
            )
        )
    return in_maps


def kernel(**inputs) -> np.ndarray:
    nc = _get_nc()
    in_maps = make_in_maps(inputs)
    res = run_bass_kernel_spmd(nc, in_maps, list(range(NCORES)))
    out = np.empty((B, N, C), np.float32)
    for core in range(NCORES):
        b, h = core // 2, core % 2
        out[b, h * NH : (h + 1) * NH] = np.asarray(
            res.results[core]["out"], dtype=np.float32
        )
    return out
